# revision 1
# baseline (speedup 1.0000x reference)
"""Bahdanau attention kernel for Trainium2 (8 NeuronCores, data-parallel over batch).

Reference computation (per batch row b):
    pq      = query @ Wq.T                       # (B, AD)
    hidden  = tanh(pq[:, None, :] + processed_memory)   # (B, T, AD)
    e       = einsum('btd,d->bt', hidden, v)     # (B, T)
    e       = where(mask, -1e30, e)
    out     = softmax(e, axis=1)

Device strategy (per core, 8 batches):
  * processed_memory is host-transposed to [b, AD, T] so AD sits on SBUF
    partitions.  The per-d "+pq" add then folds into the ScalarE tanh as a
    per-partition activation bias (free), and the v-weighted reduction over d
    becomes TensorE matmuls with a [128,1] stationary v column (M=1, free up
    to 512) accumulating in PSUM.
  * Energies strips [1, 2048] leave PSUM via a VectorE copy, then tiny
    SBUF->SBUF DMAs relayout them into an [8, T] tile (one batch per
    partition) where the masked softmax runs along the free dimension:
    exp on ScalarE, mask-multiply + row-sum fused in one
    tensor_tensor_reduce, reciprocal + scale on VectorE.
  * mask is applied multiplicatively: softmax(where(m,-1e30,e)) ==
    exp(e)*(1-m) / sum(exp(e)*(1-m)) exactly (exp(-1e30) underflows to 0,
    and |e| <= sum|v| ~ 13 so exp(e) cannot overflow in fp32).
"""

import sys

if "/opt/trn_rl_repo" not in sys.path:
    sys.path.insert(0, "/opt/trn_rl_repo")

import numpy as np

import concourse.bacc as bacc
import concourse.bass as bass
import concourse.tile as tile
from concourse import mybir
from concourse.bass_utils import run_bass_kernel_spmd

B, T, QD, AD = 64, 4096, 1024, 256
NCORES = 8
BLOC = B // NCORES  # batches per core
KB = QD // 128      # k-blocks for the pq matmul
DB = AD // 128      # d-blocks (partition blocks of AD)
F32 = mybir.dt.float32
F16 = mybir.dt.float16
U8 = mybir.dt.uint8


def build_nc() -> bass.Bass:
    # Bacc (not plain Bass): its nop/event-semaphore lowering passes are what
    # let Tile-scheduled instructions carry multiple semaphore waits.
    nc = bacc.Bacc(None, target_bir_lowering=False)

    # fp16: halves the dominant HBM stream; pm ~ N(0,1) so fp16 quantization
    # (10 mantissa bits) costs ~2e-4 rel err on the softmax output
    pm_t = nc.declare_dram_parameter("pm_t", [BLOC, AD, T], F16, isOutput=False)
    # qT[p, kb*BLOC + b] = query[b, kb*128 + p]  (host-packed, partition-major)
    qT = nc.declare_dram_parameter("qT", [128, KB * BLOC], F32, isOutput=False)
    msk = nc.declare_dram_parameter("mask", [BLOC, T], U8, isOutput=False)
    WqT = nc.declare_dram_parameter("WqT", [QD, AD], F32, isOutput=False)
    v_r = nc.declare_dram_parameter("v_r", [128, DB], F32, isOutput=False)
    # block-indicator matrices for the softmax cross-partition matmuls:
    # sel16[p, b] = 1.0 iff p // 16 == b ; sel16T is its transpose
    sel16_d = nc.declare_dram_parameter("sel16", [128, B // NCORES], F32, isOutput=False)
    sel16T_d = nc.declare_dram_parameter("sel16T", [B // NCORES, 128], F32, isOutput=False)
    out = nc.declare_dram_parameter("out", [BLOC, T], F32, isOutput=True)

    Tanh = mybir.ActivationFunctionType.Tanh
    Exp = mybir.ActivationFunctionType.Exp
    mult = mybir.AluOpType.mult
    add = mybir.AluOpType.add

    HT = 2048          # energies strip length (4 PSUM banks)
    NMM = HT // 512    # matmuls per strip per d-block
    PB = 16            # partitions per batch in the softmax layout
    PF = T // PB       # 256 free elements per partition

    with tile.TileContext(nc) as tc:
        with (
            tc.tile_pool(name="singles", bufs=1) as singles,
            tc.tile_pool(name="pm", bufs=8) as pm_pool,
            tc.tile_pool(name="hid", bufs=6) as hid_pool,
            tc.tile_pool(name="estrip", bufs=4) as estrip_pool,
            tc.tile_pool(name="epsum", bufs=2, space="PSUM") as epsum_pool,
        ):
            # ---- constant loads (wq/qt first: they gate pq -> first tanh) ----
            wq_sb = singles.tile([128, KB, AD], F32)
            nc.sync.dma_start(
                out=wq_sb, in_=WqT[:, :].rearrange("(kb p) d -> p kb d", p=128)
            )
            qt_sb = singles.tile([128, KB, BLOC], F32)
            nc.sync.dma_start(
                out=qt_sb, in_=qT[:, :].rearrange("p (kb b) -> p kb b", b=BLOC)
            )
            v_sb = singles.tile([128, DB], F32)
            nc.sync.dma_start(out=v_sb, in_=v_r[:, :])
            # fp16 copy of v for the energies matmuls: fp32 matmuls run as
            # two PE passes at ~4x the cost; tanh outputs are in [-1,1] and
            # v is small, so fp16 (10 mantissa bits) costs ~3e-4 rel err.
            v16_sb = singles.tile([128, DB], F16)
            nc.vector.tensor_copy(out=v16_sb, in_=v_sb)

            # ---- pq = Wq @ query.T, laid out [d % 128, dblk, b] ----
            pq_sb = singles.tile([128, DB, BLOC], F32)
            for d in range(DB):
                ppq = epsum_pool.tile([128, BLOC], F32, tag="ep")
                for k in range(KB):
                    nc.tensor.matmul(
                        ppq,
                        lhsT=wq_sb[:, k, d * 128 : (d + 1) * 128],
                        rhs=qt_sb[:, k, :],
                        start=(k == 0),
                        stop=(k == KB - 1),
                    )
                nc.scalar.copy(pq_sb[:, d, :], ppq)

            e2_sb = singles.tile([128, PF], F32)
            work2 = singles.tile([128, PF], F32)
            colsum = singles.tile([128, 1], F32)
            rinv_sb = singles.tile([BLOC, 1], F32)

            # ---- main loop: tanh + v-reduction ----
            for b in range(BLOC):
                hid = []
                for d in range(DB):
                    pm_sb = pm_pool.tile([128, T], F16)
                    nc.sync.dma_start(
                        out=pm_sb, in_=pm_t[b, d * 128 : (d + 1) * 128, :]
                    )
                    h = hid_pool.tile([128, T], F16)
                    nc.scalar.activation(
                        out=h,
                        in_=pm_sb,
                        func=Tanh,
                        bias=pq_sb[:, d, b : b + 1],
                        scale=1.0,
                    )
                    hid.append(h)
                for half in range(T // HT):
                    ep = epsum_pool.tile([1, HT], F32, tag="ep")
                    for c in range(NMM):
                        lo = half * HT + c * 512
                        nc.tensor.matmul(
                            ep[:, c * 512 : (c + 1) * 512],
                            lhsT=v16_sb[:, 0:1],
                            rhs=hid[0][:, lo : lo + 512],
                            start=True,
                            stop=False,
                        )
                        nc.tensor.matmul(
                            ep[:, c * 512 : (c + 1) * 512],
                            lhsT=v16_sb[:, 1:2],
                            rhs=hid[1][:, lo : lo + 512],
                            start=False,
                            stop=True,
                        )
                    es = estrip_pool.tile([1, HT], F32)
                    nc.vector.tensor_copy(out=es, in_=ep)
                    p0 = b * PB + half * (HT // PF)
                    nc.gpsimd.dma_start(
                        out=e2_sb[p0 : p0 + HT // PF, :], in_=es
                    )

            # ---- softmax-side constants ----
            # energies layout for the post pass: partition p = b*PB + q holds
            # t in [ (p%PB)*PF, ... ) of batch b = p//PB -> all 128 partitions
            # work during the softmax instead of 8.
            mask2_sb = singles.tile([128, PF], U8)
            nc.sync.dma_start(
                out=mask2_sb, in_=msk[:, :].rearrange("b (q f) -> (b q) f", f=PF)
            )
            maskz2_sb = singles.tile([128, PF], F32)
            nc.vector.tensor_scalar(
                out=maskz2_sb,
                in0=mask2_sb,
                scalar1=-1.0,
                scalar2=1.0,
                op0=mult,
                op1=add,
            )
            sel16 = singles.tile([128, BLOC], F32)
            nc.sync.dma_start(out=sel16, in_=sel16_d[:, :])
            sel16T = singles.tile([BLOC, 128], F32)
            nc.sync.dma_start(out=sel16T, in_=sel16T_d[:, :])


            # ---- masked softmax, all 128 partitions busy ----
            nc.scalar.activation(out=work2, in_=e2_sb, func=Exp)
            # (tensor_tensor_reduce is a custom ant-dve ucode op that faults
            # on this runtime — use the two standard ops instead)
            nc.vector.tensor_mul(work2, work2, maskz2_sb)
            nc.vector.reduce_sum(out=colsum, in_=work2, axis=mybir.AxisListType.X)
            # per-batch row sums: rowsum[b] = sum_p sel16[p, b] * colsum[p]
            psum_rs = epsum_pool.tile([BLOC, 1], F32, tag="ep")
            nc.tensor.matmul(psum_rs, lhsT=sel16, rhs=colsum, start=True, stop=True)
            nc.vector.reciprocal(out=rinv_sb, in_=psum_rs)
            # broadcast 1/rowsum back to the 16 partitions of each batch
            psum_ri = epsum_pool.tile([128, 1], F32, tag="ep")
            nc.tensor.matmul(psum_ri, lhsT=sel16T, rhs=rinv_sb, start=True, stop=True)
            nc.vector.tensor_scalar_mul(out=work2, in0=work2, scalar1=psum_ri)
            nc.sync.dma_start(
                out=out[:, :].rearrange("b (q f) -> (b q) f", f=PF), in_=work2
            )

    # Run the Bacc lowering passes (move_matmul_waits_to_ldweights,
    # generate_event_semaphores, alloc_regs, ...) — run_bass_via_pjrt takes
    # the module as-is and walrus rejects unlowered multi-wait instructions.
    nc.finalize()
    return nc


_CACHE: dict = {}


def _get_nc() -> bass.Bass:
    if "nc" not in _CACHE:
        _CACHE["nc"] = build_nc()
    return _CACHE["nc"]


def make_in_maps(query, processed_memory, mask, Wq, v):
    query = np.ascontiguousarray(np.asarray(query, dtype=np.float32))
    pm = np.asarray(processed_memory, dtype=np.float32)
    mask_u8 = np.asarray(mask).astype(np.uint8)
    Wq = np.asarray(Wq, dtype=np.float32)
    v = np.asarray(v, dtype=np.float32)

    WqT = np.ascontiguousarray(Wq.T)                  # (QD, AD)
    v_r = np.ascontiguousarray(v.reshape(DB, 128).T)  # (128, DB)
    sel16 = np.zeros((128, BLOC), dtype=np.float32)
    for b in range(BLOC):
        sel16[b * 16 : (b + 1) * 16, b] = 1.0
    sel16T = np.ascontiguousarray(sel16.T)

    in_maps = []
    for i in range(NCORES):
        sl = slice(i * BLOC, (i + 1) * BLOC)
        in_maps.append(
            {
                "pm_t": np.ascontiguousarray(
                    pm[sl].transpose(0, 2, 1).astype(np.float16)
                ),
                "qT": np.ascontiguousarray(
                    query[sl]
                    .T.reshape(KB, 128, BLOC)
                    .transpose(1, 0, 2)
                    .reshape(128, KB * BLOC)
                ),
                "mask": np.ascontiguousarray(mask_u8[sl]),
                "WqT": WqT,
                "v_r": v_r,
                "sel16": sel16,
                "sel16T": sel16T,
            }
        )
    return in_maps


def run_spmd(in_maps, **kwargs):
    return run_bass_kernel_spmd(_get_nc(), in_maps, list(range(NCORES)), **kwargs)


def kernel(query, processed_memory, mask, Wq, v) -> np.ndarray:
    in_maps = make_in_maps(query, processed_memory, mask, Wq, v)
    res = run_spmd(in_maps)
    return np.concatenate(
        [res.results[i]["out"] for i in range(NCORES)], axis=0
    ).astype(np.float32)



# revision 2
# speedup vs baseline: 1.6081x; 1.6081x over previous
"""Bahdanau attention kernel for Trainium2 (8 NeuronCores, data-parallel over batch).

Reference computation (per batch row b):
    pq      = query @ Wq.T                       # (B, AD)
    hidden  = tanh(pq[:, None, :] + processed_memory)   # (B, T, AD)
    e       = einsum('btd,d->bt', hidden, v)     # (B, T)
    e       = where(mask, -1e30, e)
    out     = softmax(e, axis=1)

Key observation: ~50% of positions are masked (mask True -> softmax weight
exactly 0), so the host gathers only the unmasked columns of
processed_memory per batch (max count 2126 of 4096 for the reference data)
into a compact [AD, Tc=2176] slab, and scatters the compact softmax back
into the full [B, T] output (zeros at masked positions).  This halves HBM
traffic, tanh work and matmul work on the device.

Device strategy (per core, 8 batches):
  * gathered pm is host-transposed to [b, 2, 128, Tc] fp16 (AD=256 split in
    two partition blocks).  The per-d "+pq" add folds into the ScalarE tanh
    as a per-partition activation bias, v-weighted reduction over d runs on
    TensorE.
  * Energies accumulate directly into a PSUM tile [8, 2048] (+[8, 128]
    tail) with batch = partition: each matmul uses a one-hot stationary
    [128, 8] whose column b holds v (other columns zero), so it adds
    v.h on row b and +0 on the other rows.  No PSUM->SBUF strip copies, no
    SBUF relayout DMAs.
  * Pad columns (count_b..Tc) hold -10*sign(v_d): tanh saturates to
    -sign(v_d), so each pad contributes exp(-sum|v|) ~ 3e-6 to the softmax
    sum (rel err ~1e-7); pad outputs are discarded by the host scatter.
  * Softmax tail with batch on partitions: exp reads PSUM directly,
    row-sum + reciprocal + rescale on DVE, direct DMA to out[b, t].
"""

import sys

if "/opt/trn_rl_repo" not in sys.path:
    sys.path.insert(0, "/opt/trn_rl_repo")

import numpy as np

import concourse.bacc as bacc
import concourse.bass as bass
import concourse.tile as tile
from concourse import mybir
from concourse.bass_utils import run_bass_kernel_spmd

B, T, QD, AD = 64, 4096, 1024, 256
NCORES = 8
BLOC = B // NCORES  # batches per core
KB = QD // 128      # k-blocks for the pq matmul
DB = AD // 128      # d-blocks (partition blocks of AD)
F32 = mybir.dt.float32
F16 = mybir.dt.float16

MAIN = 2048         # main energies region (4 PSUM banks of 512 fp32)
TAIL = 128          # tail region (counts range ~1984..2126 < 2176)
TC = MAIN + TAIL    # compact (gathered) time extent per batch
NCH = MAIN // 512   # 512-col chunks in the main region


def build_nc() -> bass.Bass:
    # Bacc (not plain Bass): its nop/event-semaphore lowering passes are what
    # let Tile-scheduled instructions carry multiple semaphore waits.
    nc = bacc.Bacc(None, target_bir_lowering=False)

    pm_g = nc.declare_dram_parameter("pm_g", [BLOC, DB, 128, TC], F16, isOutput=False)
    # qT16[p, kb*BLOC + b] = query[b, kb*128 + p]  (host-packed, partition-major)
    qT16 = nc.declare_dram_parameter("qT16", [128, KB * BLOC], F16, isOutput=False)
    WqT16 = nc.declare_dram_parameter("WqT16", [QD, AD], F16, isOutput=False)
    # one-hot stationaries: oh[p, db, b, m] = v[db*128+p] if m == b else 0
    ohd = nc.declare_dram_parameter("oh", [128, DB, BLOC, BLOC], F16, isOutput=False)
    out = nc.declare_dram_parameter("out", [BLOC, TC], F32, isOutput=True)
    rsum = nc.declare_dram_parameter("rsum", [BLOC, 1], F32, isOutput=True)

    Tanh = mybir.ActivationFunctionType.Tanh
    Exp = mybir.ActivationFunctionType.Exp

    with tile.TileContext(nc) as tc:
        with (
            tc.tile_pool(name="singles", bufs=1) as singles,
            tc.tile_pool(name="pm", bufs=4) as pm_pool,
            tc.tile_pool(name="hid", bufs=6) as hid_pool,
            tc.tile_pool(name="epsum", bufs=1, space="PSUM") as epsum_pool,
            tc.tile_pool(name="ppsum", bufs=2, space="PSUM") as ppsum_pool,
        ):
            # ---- constant loads (wq/qt first: they gate pq -> first tanh) ----
            wq_sb = singles.tile([128, KB, AD], F16)
            nc.sync.dma_start(
                out=wq_sb, in_=WqT16[:, :].rearrange("(kb p) d -> p kb d", p=128)
            )
            qt_sb = singles.tile([128, KB, BLOC], F16)
            nc.sync.dma_start(
                out=qt_sb, in_=qT16[:, :].rearrange("p (kb b) -> p kb b", b=BLOC)
            )
            oh_sb = singles.tile([128, DB, BLOC, BLOC], F16)
            nc.sync.dma_start(out=oh_sb, in_=ohd[:, :, :, :])

            # ---- pq = Wq @ query.T, laid out [d % 128, dblk, b] ----
            pq_sb = singles.tile([128, DB, BLOC], F32)
            for d in range(DB):
                ppq = ppsum_pool.tile([128, BLOC], F32, tag="ppq")
                for k in range(KB):
                    nc.tensor.matmul(
                        ppq,
                        lhsT=wq_sb[:, k, d * 128 : (d + 1) * 128],
                        rhs=qt_sb[:, k, :],
                        start=(k == 0),
                        stop=(k == KB - 1),
                    )
                nc.scalar.copy(pq_sb[:, d, :], ppq)

            # ---- energies PSUM accumulators (batch = partition row) ----
            ep = epsum_pool.tile([BLOC, MAIN], F32, tag="ep")
            ept = epsum_pool.tile([BLOC, TAIL], F32, tag="ept")

            # ---- main loop: tanh + one-hot v-reduction ----
            for b in range(BLOC):
                pm_sb = pm_pool.tile([128, DB, TC], F16)
                nc.sync.dma_start(
                    out=pm_sb, in_=pm_g[b].rearrange("db p t -> p db t")
                )
                for d in range(DB):
                    h = hid_pool.tile([128, TC], F16)
                    nc.scalar.activation(
                        out=h,
                        in_=pm_sb[:, d, :],
                        func=Tanh,
                        bias=pq_sb[:, d, b : b + 1],
                        scale=1.0,
                    )
                    first = b == 0 and d == 0
                    last = b == BLOC - 1 and d == DB - 1
                    for c in range(NCH):
                        nc.tensor.matmul(
                            ep[:, c * 512 : (c + 1) * 512],
                            lhsT=oh_sb[:, d, b, :],
                            rhs=h[:, c * 512 : (c + 1) * 512],
                            start=first,
                            stop=last,
                        )
                    nc.tensor.matmul(
                        ept,
                        lhsT=oh_sb[:, d, b, :],
                        rhs=h[:, MAIN:TC],
                        start=first,
                        stop=last,
                    )

            # ---- softmax tail: batch already on partitions ----
            work = singles.tile([BLOC, MAIN], F32)
            workt = singles.tile([BLOC, TAIL], F32)
            nc.scalar.activation(out=work, in_=ep, func=Exp)
            nc.scalar.activation(out=workt, in_=ept, func=Exp)
            cs = singles.tile([BLOC, 1], F32)
            cst = singles.tile([BLOC, 1], F32)
            nc.vector.reduce_sum(out=cs, in_=work, axis=mybir.AxisListType.X)
            nc.vector.reduce_sum(out=cst, in_=workt, axis=mybir.AxisListType.X)
            s_sb = singles.tile([BLOC, 1], F32)
            nc.vector.tensor_add(s_sb, cs, cst)
            nc.sync.dma_start(out=rsum[:, :], in_=s_sb)
            rinv = singles.tile([BLOC, 1], F32)
            nc.vector.reciprocal(out=rinv, in_=s_sb)
            nc.vector.tensor_scalar_mul(out=work, in0=work, scalar1=rinv)
            nc.vector.tensor_scalar_mul(out=workt, in0=workt, scalar1=rinv)
            nc.sync.dma_start(out=out[:, 0:MAIN], in_=work)
            nc.sync.dma_start(out=out[:, MAIN:TC], in_=workt)

    # Run the Bacc lowering passes (move_matmul_waits_to_ldweights,
    # generate_event_semaphores, alloc_regs, ...) — run_bass_via_pjrt takes
    # the module as-is and walrus rejects unlowered multi-wait instructions.
    nc.finalize()
    return nc


_CACHE: dict = {}


def _get_nc() -> bass.Bass:
    if "nc" not in _CACHE:
        _CACHE["nc"] = build_nc()
    return _CACHE["nc"]


def _pack_qT(query: np.ndarray) -> np.ndarray:
    return np.ascontiguousarray(
        query.T.reshape(KB, 128, BLOC * NCORES)  # [kb, p, B]
    )


def _prep(query, processed_memory, mask, Wq, v):
    """Build per-core input maps + scatter metadata for one gather pass set."""
    query = np.asarray(query, dtype=np.float32)
    pm = np.asarray(processed_memory)
    mask_b = np.asarray(mask).astype(bool)
    Wq = np.asarray(Wq, dtype=np.float32)
    v = np.asarray(v, dtype=np.float32)

    WqT16 = np.ascontiguousarray(Wq.T.astype(np.float16))      # (QD, AD)
    v16 = v.astype(np.float16)
    # oh[p, db, b, m] = v[db*128+p] iff m == b
    oh = np.zeros((128, DB, BLOC, BLOC), dtype=np.float16)
    for b_ in range(BLOC):
        oh[:, :, b_, b_] = v16.reshape(DB, 128).T
    padcol = (-10.0 * np.sign(v)).astype(np.float16)           # (AD,)

    idxs = [np.flatnonzero(~mask_b[gb]) for gb in range(B)]
    counts = np.array([len(ix) for ix in idxs])
    npass = max(1, int(np.ceil(counts.max() / TC)))

    qfull = query.T.reshape(KB, 128, B).transpose(1, 0, 2)     # (128, KB, B)

    pass_maps = []
    for p_ in range(npass):
        in_maps = []
        for i in range(NCORES):
            arr = np.empty((BLOC, DB, 128, TC), dtype=np.float16)
            arr[:] = padcol.reshape(1, DB, 128, 1)
            for b_ in range(BLOC):
                gb = i * BLOC + b_
                ix = idxs[gb][p_ * TC : (p_ + 1) * TC]
                if len(ix):
                    g = pm[gb, ix, :].astype(np.float16)       # (cnt, AD)
                    arr[b_, :, :, : len(ix)] = g.T.reshape(DB, 128, len(ix))
            sl = slice(i * BLOC, (i + 1) * BLOC)
            in_maps.append(
                {
                    "pm_g": arr,
                    "qT16": np.ascontiguousarray(
                        qfull[:, :, sl].reshape(128, KB * BLOC).astype(np.float16)
                    ),
                    "WqT16": WqT16,
                    "oh": oh,
                }
            )
        pass_maps.append(in_maps)
    return pass_maps, idxs, counts, npass


def run_spmd(in_maps, **kwargs):
    return run_bass_kernel_spmd(_get_nc(), in_maps, list(range(NCORES)), **kwargs)


def run_full(inputs: dict, **kwargs):
    """Run the full pipeline; returns (full_output, last_spmd_result)."""
    pass_maps, idxs, counts, npass = _prep(**inputs)
    results = []
    res = None
    for p_ in range(npass):
        res = run_spmd(pass_maps[p_], **kwargs)
        kwargs.pop("trace", None)  # only trace the first pass
        outs = np.concatenate(
            [res.results[i]["out"] for i in range(NCORES)], axis=0
        )  # (B, TC)
        sums = np.concatenate(
            [res.results[i]["rsum"] for i in range(NCORES)], axis=0
        )[:, 0]  # (B,)
        results.append((outs, sums))

    full = np.zeros((B, T), dtype=np.float32)
    for gb in range(B):
        cnt = counts[gb]
        if cnt == 0:
            full[gb, :] = 1.0 / T  # all masked -> uniform softmax
            continue
        if npass == 1:
            full[gb, idxs[gb]] = results[0][0][gb, :cnt]
        else:
            stot = sum(s[gb] for _, s in results if True)
            for p_ in range(npass):
                lo = p_ * TC
                ix = idxs[gb][lo : lo + TC]
                if len(ix):
                    o, s = results[p_]
                    full[gb, ix] = o[gb, : len(ix)] * (s[gb] / stot)
    return full, res


def kernel(query, processed_memory, mask, Wq, v) -> np.ndarray:
    full, _ = run_full(
        dict(query=query, processed_memory=processed_memory, mask=mask, Wq=Wq, v=v)
    )
    return full


# revision 5
# speedup vs baseline: 2.0015x; 1.2447x over previous
"""Bahdanau attention kernel for Trainium2 (8 NeuronCores, data-parallel over batch).

Reference computation (per batch row b):
    pq      = query @ Wq.T                       # (B, AD)
    hidden  = tanh(pq[:, None, :] + processed_memory)   # (B, T, AD)
    e       = einsum('btd,d->bt', hidden, v)     # (B, T)
    e       = where(mask, -1e30, e)
    out     = softmax(e, axis=1)

Key observation: ~50% of positions are masked (mask True -> softmax weight
exactly 0), so the host gathers only the unmasked columns of
processed_memory per batch (max count 2126 of 4096 for the reference data)
into a compact [AD, Tc=2176] slab, and scatters the compact softmax back
into the full [B, T] output (zeros at masked positions).  This halves HBM
traffic, tanh work and matmul work on the device.

Device strategy (per core, 8 batches):
  * gathered pm is host-transposed to [b, 2, 128, Tc] fp16 (AD=256 split in
    two partition blocks).  The per-d "+pq" add folds into the ScalarE tanh
    as a per-partition activation bias, v-weighted reduction over d runs on
    TensorE.
  * Energies accumulate directly into a PSUM tile [8, 2048] (+[8, 128]
    tail) with batch = partition: each matmul uses a one-hot stationary
    [128, 8] whose column b holds v (other columns zero), so it adds
    v.h on row b and +0 on the other rows.  No PSUM->SBUF strip copies, no
    SBUF relayout DMAs.
  * Pad columns (count_b..Tc) hold -10*sign(v_d): tanh saturates to
    -sign(v_d), so each pad contributes exp(-sum|v|) ~ 3e-6 to the softmax
    sum (rel err ~1e-7); pad outputs are discarded by the host scatter.
  * Softmax tail with batch on partitions: exp reads PSUM directly,
    row-sum + reciprocal + rescale on DVE, direct DMA to out[b, t].
"""

import sys

if "/opt/trn_rl_repo" not in sys.path:
    sys.path.insert(0, "/opt/trn_rl_repo")

import numpy as np

import concourse.bacc as bacc
import concourse.bass as bass
import concourse.tile as tile
from concourse import mybir
from concourse.bass_utils import run_bass_kernel_spmd

B, T, QD, AD = 64, 4096, 1024, 256
NCORES = 8
BLOC = B // NCORES  # batches per core
KB = QD // 128      # k-blocks for the pq matmul
DB = AD // 128      # d-blocks (partition blocks of AD)
F32 = mybir.dt.float32
F16 = mybir.dt.float16

MAIN = 2048         # main energies region (4 PSUM banks of 512 fp32)
TAIL = 128          # tail region (counts range ~1984..2126 < 2176)
TC = MAIN + TAIL    # compact (gathered) time extent per batch
NCH = MAIN // 512   # 512-col chunks in the main region


def build_nc() -> bass.Bass:
    # Bacc (not plain Bass): its nop/event-semaphore lowering passes are what
    # let Tile-scheduled instructions carry multiple semaphore waits.
    nc = bacc.Bacc(None, target_bir_lowering=False)

    pm_g = nc.declare_dram_parameter("pm_g", [BLOC, DB, 128, TC], F16, isOutput=False)
    # qT16[p, kb*BLOC + b] = query[b, kb*128 + p]  (host-packed, partition-major)
    qT16 = nc.declare_dram_parameter("qT16", [128, KB * BLOC], F16, isOutput=False)
    WqT16 = nc.declare_dram_parameter("WqT16", [QD, AD], F16, isOutput=False)
    # one-hot stationaries: oh[p, db, b, m] = v[db*128+p] if m == b else 0
    ohd = nc.declare_dram_parameter("oh", [128, DB, BLOC, BLOC], F16, isOutput=False)
    out = nc.declare_dram_parameter("out", [BLOC, TC], F32, isOutput=True)
    rsum = nc.declare_dram_parameter("rsum", [BLOC, 1], F32, isOutput=True)

    Tanh = mybir.ActivationFunctionType.Tanh
    Exp = mybir.ActivationFunctionType.Exp

    # energy matmul chunks: 512-col chunks (one PSUM bank each) + 128 tail
    chunks = [(c * 512, (c + 1) * 512) for c in range(NCH)] + [(MAIN, TC)]

    with tile.TileContext(nc) as tc:
        with (
            tc.tile_pool(name="singles", bufs=1) as singles,
            tc.tile_pool(name="pm", bufs=8) as pm_pool,
            tc.tile_pool(name="hid", bufs=6) as hid_pool,
            tc.tile_pool(name="epsum", bufs=1, space="PSUM") as epsum_pool,
            tc.tile_pool(name="ppsum", bufs=2, space="PSUM") as ppsum_pool,
        ):
            # ---- constant loads (wq/qt first: they gate pq -> first tanh;
            # then the first pm tile so tanh b0/d0 starts ASAP) ----
            wq_sb = singles.tile([128, KB, AD], F16)
            nc.sync.dma_start(
                out=wq_sb, in_=WqT16[:, :].rearrange("(kb p) d -> p kb d", p=128)
            )
            qt_sb = singles.tile([128, KB, BLOC], F16)
            nc.sync.dma_start(
                out=qt_sb, in_=qT16[:, :].rearrange("p (kb b) -> p kb b", b=BLOC)
            )
            pm_sbs = {}
            for b, d in ((0, 0), (0, 1)):
                pm_sbs[(b, d)] = pm_pool.tile([128, TC], F16, name="pm_sb", tag="pm_sb")
                nc.sync.dma_start(out=pm_sbs[(b, d)], in_=pm_g[b, d])
            oh_sb = singles.tile([128, DB, BLOC, BLOC], F16)
            nc.sync.dma_start(out=oh_sb, in_=ohd[:, :, :, :])
            for b in range(1, BLOC):
                for d in range(DB):
                    pm_sbs[(b, d)] = pm_pool.tile([128, TC], F16, name="pm_sb", tag="pm_sb")
                    nc.sync.dma_start(out=pm_sbs[(b, d)], in_=pm_g[b, d])

            # ---- pq = Wq @ query.T, laid out [d % 128, dblk, b] ----
            pq_sb = singles.tile([128, DB, BLOC], F32)
            for d in range(DB):
                ppq = ppsum_pool.tile([128, BLOC], F32, tag="ppq")
                for k in range(KB):
                    nc.tensor.matmul(
                        ppq,
                        lhsT=wq_sb[:, k, d * 128 : (d + 1) * 128],
                        rhs=qt_sb[:, k, :],
                        start=(k == 0),
                        stop=(k == KB - 1),
                    )
                nc.scalar.copy(pq_sb[:, d, :], ppq)

            # ---- energies PSUM accumulator (batch = partition row) ----
            # [8, 2176] fp32 spans 4.25 banks; every matmul chunk below stays
            # inside a single 512-fp32 bank region.
            ep = epsum_pool.tile([BLOC, TC], F32, tag="ep")

            # ---- main loop: tanh + one-hot v-reduction ----
            for b in range(BLOC):
                for d in range(DB):
                    h = hid_pool.tile([128, TC], F16)
                    nc.scalar.activation(
                        out=h,
                        in_=pm_sbs[(b, d)],
                        func=Tanh,
                        bias=pq_sb[:, d, b : b + 1],
                        scale=1.0,
                    )
                    first = b == 0 and d == 0
                    last = b == BLOC - 1 and d == DB - 1
                    for lo, hi in chunks:
                        nc.tensor.matmul(
                            ep[:, lo:hi],
                            lhsT=oh_sb[:, d, b, :],
                            rhs=h[:, lo:hi],
                            start=first,
                            stop=last,
                        )

            # ---- softmax tail: batch already on partitions; exp reads PSUM
            # directly and its row-sum is fused via accum_out ----
            work = singles.tile([BLOC, TC], F32)
            cs = singles.tile([BLOC, 1], F32)
            nc.scalar.activation(out=work, in_=ep, func=Exp, accum_out=cs)
            nc.sync.dma_start(out=rsum[:, :], in_=cs)
            rinv = singles.tile([BLOC, 1], F32)
            nc.vector.reciprocal(out=rinv, in_=cs)
            nc.vector.tensor_scalar_mul(out=work, in0=work, scalar1=rinv)
            nc.sync.dma_start(out=out[:, :], in_=work)

    # Run the Bacc lowering passes (move_matmul_waits_to_ldweights,
    # generate_event_semaphores, alloc_regs, ...) — run_bass_via_pjrt takes
    # the module as-is and walrus rejects unlowered multi-wait instructions.
    nc.finalize()
    return nc


_CACHE: dict = {}


def _get_nc() -> bass.Bass:
    if "nc" not in _CACHE:
        _CACHE["nc"] = build_nc()
    return _CACHE["nc"]


def _pack_qT(query: np.ndarray) -> np.ndarray:
    return np.ascontiguousarray(
        query.T.reshape(KB, 128, BLOC * NCORES)  # [kb, p, B]
    )


def _prep(query, processed_memory, mask, Wq, v):
    """Build per-core input maps + scatter metadata for one gather pass set."""
    query = np.asarray(query, dtype=np.float32)
    pm = np.asarray(processed_memory)
    mask_b = np.asarray(mask).astype(bool)
    Wq = np.asarray(Wq, dtype=np.float32)
    v = np.asarray(v, dtype=np.float32)

    WqT16 = np.ascontiguousarray(Wq.T.astype(np.float16))      # (QD, AD)
    v16 = v.astype(np.float16)
    # oh[p, db, b, m] = v[db*128+p] iff m == b
    oh = np.zeros((128, DB, BLOC, BLOC), dtype=np.float16)
    for b_ in range(BLOC):
        oh[:, :, b_, b_] = v16.reshape(DB, 128).T
    padcol = (-10.0 * np.sign(v)).astype(np.float16)           # (AD,)

    idxs = [np.flatnonzero(~mask_b[gb]) for gb in range(B)]
    counts = np.array([len(ix) for ix in idxs])
    npass = max(1, int(np.ceil(counts.max() / TC)))

    qfull = query.T.reshape(KB, 128, B).transpose(1, 0, 2)     # (128, KB, B)

    pass_maps = []
    for p_ in range(npass):
        in_maps = []
        for i in range(NCORES):
            arr = np.empty((BLOC, DB, 128, TC), dtype=np.float16)
            arr[:] = padcol.reshape(1, DB, 128, 1)
            for b_ in range(BLOC):
                gb = i * BLOC + b_
                ix = idxs[gb][p_ * TC : (p_ + 1) * TC]
                if len(ix):
                    g = pm[gb, ix, :].astype(np.float16)       # (cnt, AD)
                    arr[b_, :, :, : len(ix)] = g.T.reshape(DB, 128, len(ix))
            sl = slice(i * BLOC, (i + 1) * BLOC)
            in_maps.append(
                {
                    "pm_g": arr,
                    "qT16": np.ascontiguousarray(
                        qfull[:, :, sl].reshape(128, KB * BLOC).astype(np.float16)
                    ),
                    "WqT16": WqT16,
                    "oh": oh,
                }
            )
        pass_maps.append(in_maps)
    return pass_maps, idxs, counts, npass


def run_spmd(in_maps, **kwargs):
    return run_bass_kernel_spmd(_get_nc(), in_maps, list(range(NCORES)), **kwargs)


def run_full(inputs: dict, **kwargs):
    """Run the full pipeline; returns (full_output, last_spmd_result)."""
    pass_maps, idxs, counts, npass = _prep(**inputs)
    results = []
    res = None
    for p_ in range(npass):
        res = run_spmd(pass_maps[p_], **kwargs)
        kwargs.pop("trace", None)  # only trace the first pass
        outs = np.concatenate(
            [res.results[i]["out"] for i in range(NCORES)], axis=0
        )  # (B, TC)
        sums = np.concatenate(
            [res.results[i]["rsum"] for i in range(NCORES)], axis=0
        )[:, 0]  # (B,)
        results.append((outs, sums))

    full = np.zeros((B, T), dtype=np.float32)
    for gb in range(B):
        cnt = counts[gb]
        if cnt == 0:
            full[gb, :] = 1.0 / T  # all masked -> uniform softmax
            continue
        if npass == 1:
            full[gb, idxs[gb]] = results[0][0][gb, :cnt]
        else:
            stot = sum(s[gb] for _, s in results if True)
            for p_ in range(npass):
                lo = p_ * TC
                ix = idxs[gb][lo : lo + TC]
                if len(ix):
                    o, s = results[p_]
                    full[gb, ix] = o[gb, : len(ix)] * (s[gb] / stot)
    return full, res


def kernel(query, processed_memory, mask, Wq, v) -> np.ndarray:
    full, _ = run_full(
        dict(query=query, processed_memory=processed_memory, mask=mask, Wq=Wq, v=v)
    )
    return full
